# revision 9
# baseline (speedup 1.0000x reference)
"""MLA (multi-head latent attention) forward kernel for Trainium2, 8 NeuronCores.

Sharding: 8 cores = 2 (batch) x 4 (head-groups of 10 heads).
Each core computes, for its batch b and its 10 heads:
  - full fused down-projection a = x @ w_a (transposed-activation layout)
  - rmsnorm of q_lora / kv_lora segments, rope of k_pe
  - q/kv up-projections for its heads, causal attention, and the partial
    o-projection (w_o rows of its heads).  Host sums the 4 partials per batch.

Device layout notes:
  - activations are kept transposed ([feature, seq]) so weights act as the
    stationary lhsT operand of the PE in their natural [in, out] orientation.
  - attention computes scoresT [keys, q]; softmax runs without max-subtraction
    (scores are bounded by construction), masking is a binary multiply on the
    exp'd probabilities, and sum-of-exp comes from a ones-column appended to V
    in the AV matmul.  Per-row 1/sum is applied on PSUM eviction.
  - normalized aT and qT round-trip through DRAM so SBUF tile-pool lifetimes
    nest properly (pool releases must be LIFO).
"""

import math
import sys
from dataclasses import dataclass

if "/opt/trn_rl_repo" not in sys.path:
    sys.path.insert(0, "/opt/trn_rl_repo")

import ml_dtypes
import numpy as np

BF16 = ml_dtypes.bfloat16


@dataclass(frozen=True)
class Cfg:
    HID: int = 5120
    S: int = 2048
    QLR: int = 1536
    KVLR: int = 512
    DN: int = 128
    DR: int = 64
    DV: int = 128
    HPC: int = 10          # heads per core
    CHUNK: int = 512       # q-position chunk (PSUM bank width)
    SUPER: int = 1024      # x super-chunk resident in SBUF during phase A
    EPS: float = 1e-6
    THETA: float = 10000.0

    @property
    def DQK(self):
        return self.DN + self.DR

    @property
    def PEH(self):
        return self.DR // 2


FULL = Cfg()


def build_program(c: Cfg):
    import contextlib

    import concourse.bass as bass  # noqa: F401
    import concourse.mybir as mybir
    import concourse.tile as tile
    from concourse import bacc
    from concourse.masks import make_identity

    dt = mybir.dt
    BF = dt.bfloat16
    F32 = dt.float32
    Alu = mybir.AluOpType
    Act = mybir.ActivationFunctionType

    KT_HID = c.HID // 128
    KT_Q = c.QLR // 128
    KT_KV = c.KVLR // 128
    NQC = c.S // c.CHUNK
    SUPER = min(c.SUPER, c.S)
    NSC = c.S // SUPER
    QPS = SUPER // c.CHUNK
    ST = c.S // 128
    H = c.HPC
    TPC = c.CHUNK // 128            # 128-tiles per chunk (4)
    QROWS = H * (c.DN + c.DR)
    MT_QN = H * c.DN // 128
    MT_QP = H * c.DR // 128
    KROWS = H * c.DN
    VCOLS = H * c.DV
    OROWS = H * c.DV
    MT_O = c.HID // 128
    ACOLS = c.QLR + c.KVLR + c.DR
    SCALE = 1.0 / math.sqrt(c.DQK)

    assert c.DN == 128 and c.DV == 128 and c.DR == 64 and H % 2 == 0

    nc = bacc.Bacc("TRN2")
    xT = nc.dram_tensor("xT", [c.HID, c.S], BF, kind="ExternalInput")
    w_a = nc.dram_tensor("w_a", [c.HID, ACOLS], BF, kind="ExternalInput")
    w_qb = nc.dram_tensor("w_qb", [c.QLR, QROWS], BF, kind="ExternalInput")
    w_kvb = nc.dram_tensor("w_kvb", [c.KVLR, KROWS + VCOLS], BF, kind="ExternalInput")
    w_o = nc.dram_tensor("w_o", [OROWS, c.HID], BF, kind="ExternalInput")
    cosT = nc.dram_tensor("cosT", [128, c.S], BF, kind="ExternalInput")
    sinT = nc.dram_tensor("sinT", [128, c.S], BF, kind="ExternalInput")
    lnq = nc.dram_tensor("lnq", [128, KT_Q], F32, kind="ExternalInput")
    lnkv = nc.dram_tensor("lnkv", [128, KT_KV], F32, kind="ExternalInput")
    maskm = nc.dram_tensor("maskm", [128, TPC, c.CHUNK], BF, kind="ExternalInput")
    outT = nc.dram_tensor("outT", [c.HID, c.S], F32, kind="ExternalOutput")
    qTs = nc.dram_tensor("qTs", [QROWS, c.S], BF, kind="Internal")
    aTs = nc.dram_tensor("aTs", [c.QLR + c.KVLR, c.S], BF, kind="Internal")

    xT_r = xT.ap().rearrange("(t p) (n s) -> n p t s", p=128, s=SUPER)
    w_a_r = w_a.ap().rearrange("(t p) m -> p t m", p=128)
    w_qb_r = w_qb.ap().rearrange("(t p) m -> p t m", p=128)
    w_kvb_r = w_kvb.ap().rearrange("(t p) m -> p t m", p=128)
    w_o_r = w_o.ap().rearrange("(t p) m -> p t m", p=128)
    aTs_r = aTs.ap().rearrange("(t p) s -> p t s", p=128)
    qTs_ap = qTs.ap()
    outT_ap = outT.ap()

    def emit_rope(nc, pool, dst64, src64, cos_ap, sin_ap, W, p0=0):
        # cos_ap/sin_ap are [128, W] (table replicated every PEH partitions);
        # slices are taken at each operand's base partition because DVE
        # tensor_tensor requires equal base partitions for SBUF inputs.
        ph = c.PEH
        t1, t2 = src64[0:ph], src64[ph:2 * ph]
        d1, d2 = dst64[0:ph], dst64[ph:2 * ph]
        c1, s1 = cos_ap[p0:p0 + ph], sin_ap[p0:p0 + ph]
        c2, s2 = cos_ap[p0 + ph:p0 + 2 * ph], sin_ap[p0 + ph:p0 + 2 * ph]
        ra = pool.tile([ph, W], F32, tag="rope_a", name="rope_a")
        rb = pool.tile([ph, W], F32, tag="rope_b", name="rope_b")
        nc.vector.tensor_tensor(out=ra, in0=t1, in1=c1, op=Alu.mult)
        nc.vector.tensor_tensor(out=rb, in0=t2, in1=s2, op=Alu.mult)
        nc.vector.tensor_tensor(out=d1, in0=ra, in1=rb, op=Alu.subtract)
        nc.vector.tensor_tensor(out=ra, in0=t2, in1=c2, op=Alu.mult)
        nc.vector.tensor_tensor(out=rb, in0=t1, in1=s1, op=Alu.mult)
        nc.vector.tensor_tensor(out=d2, in0=ra, in1=rb, op=Alu.add)

    with tile.TileContext(nc, pool_alloc_mode="queue") as tc:
        with contextlib.ExitStack() as top:
            pers = top.enter_context(tc.tile_pool(name="pers", bufs=1))
            cos_sb = pers.tile([128, c.S], BF, tag="cos_sb")
            sin_sb = pers.tile([128, c.S], BF, tag="sin_sb")
            lnq_sb = pers.tile([128, KT_Q], F32, tag="lnq_sb")
            lnkv_sb = pers.tile([128, KT_KV], F32, tag="lnkv_sb")
            mask_sb = pers.tile([128, TPC, c.CHUNK], BF, tag="mask_sb")
            ident = pers.tile([128, 128], BF, tag="ident")
            ones_f = pers.tile([1, 128], F32, tag="ones_f")
            ones_c = pers.tile([128, 1], BF, tag="ones_c")
            eps_sb = pers.tile([1, 1], F32, tag="eps_sb")
            nc.vector.memset(eps_sb, c.EPS)
            kpe = pers.tile([c.DR, c.S], BF, tag="kpe")
            nc.sync.dma_start(out=cos_sb, in_=cosT.ap())
            nc.sync.dma_start(out=sin_sb, in_=sinT.ap())
            nc.sync.dma_start(out=lnq_sb, in_=lnq.ap())
            nc.sync.dma_start(out=lnkv_sb, in_=lnkv.ap())
            nc.sync.dma_start(out=mask_sb, in_=maskm.ap())
            make_identity(nc, ident)
            nc.vector.memset(ones_f, 1.0)
            nc.vector.memset(ones_c, 1.0)

            # ---------------- phase A: a-proj + rmsnorm + k_pe rope ----------
            with contextlib.ExitStack() as st:
                pax = st.enter_context(tc.tile_pool(name="pax", bufs=1))
                paw = st.enter_context(tc.tile_pool(name="paw", bufs=2))
                pat = st.enter_context(tc.tile_pool(name="pat", bufs=2))
                paa = st.enter_context(tc.tile_pool(name="paa", bufs=1))
                paps = st.enter_context(
                    tc.tile_pool(name="paps", bufs=2, space="PSUM"))
                pssq = st.enter_context(
                    tc.tile_pool(name="pssq", bufs=1, space="PSUM"))
                pbc = st.enter_context(
                    tc.tile_pool(name="pbc", bufs=2, space="PSUM"))

                mtiles = ([("q", i) for i in range(KT_Q)]
                          + [("kv", i) for i in range(KT_KV)]
                          + [("pe", 0)])
                for sc in range(NSC):
                    x_sb = pax.tile([128, KT_HID, SUPER], BF, tag="x_sb")
                    nc.sync.dma_start(out=x_sb, in_=xT_r[sc])
                    aq_c = paa.tile([128, KT_Q, SUPER], BF, tag="aq_c")
                    akv_c = paa.tile([128, KT_KV, SUPER], BF, tag="akv_c")
                    ssq_q = pssq.tile([1, SUPER], F32, tag="ssq_q")
                    ssq_kv = pssq.tile([1, SUPER], F32, tag="ssq_kv")
                    kperaw = pat.tile([c.DR, SUPER], BF, tag="kperaw")
                    for seg, ti in mtiles:
                        if seg == "q":
                            m0, mw = ti * 128, 128
                        elif seg == "kv":
                            m0, mw = c.QLR + ti * 128, 128
                        else:
                            m0, mw = c.QLR + c.KVLR, c.DR
                        wa_sb = paw.tile([128, KT_HID, 128], BF, tag="wa_sb")
                        nc.sync.dma_start(
                            out=wa_sb[:, :, :mw], in_=w_a_r[:, :, m0:m0 + mw])
                        for qi in range(QPS):
                            cs = slice(qi * c.CHUNK, (qi + 1) * c.CHUNK)
                            ps = paps.tile([128, c.CHUNK], F32, tag="aps")
                            for kt in range(KT_HID):
                                nc.tensor.matmul(
                                    ps[:mw], wa_sb[:, kt, :mw],
                                    x_sb[:, kt, qi * c.CHUNK:(qi + 1) * c.CHUNK],
                                    start=(kt == 0), stop=(kt == KT_HID - 1))
                            if seg == "q":
                                nc.scalar.copy(aq_c[:, ti, cs], ps)
                                sq = pat.tile([128, c.CHUNK], BF, tag="sq",
                                              bufs=3)
                                nc.scalar.square(sq, ps)
                                nc.tensor.matmul(
                                    ssq_q[:, cs], ones_c, sq,
                                    start=(ti == 0), stop=(ti == KT_Q - 1))
                            elif seg == "kv":
                                nc.scalar.copy(akv_c[:, ti, cs], ps)
                                sq = pat.tile([128, c.CHUNK], BF, tag="sq",
                                              bufs=3)
                                nc.scalar.square(sq, ps)
                                nc.tensor.matmul(
                                    ssq_kv[:, cs], ones_c, sq,
                                    start=(ti == 0), stop=(ti == KT_KV - 1))
                            else:
                                nc.scalar.copy(kperaw[:, cs], ps[:mw])
                    # normalize + rope + spill this super-chunk
                    for qi in range(QPS):
                        col = sc * SUPER + qi * c.CHUNK
                        cs = slice(qi * c.CHUNK, (qi + 1) * c.CHUNK)
                        for seg, ssq, ln_sb, ktn, denom, dst in (
                                ("q", ssq_q, lnq_sb, KT_Q, c.QLR, aq_c),
                                ("kv", ssq_kv, lnkv_sb, KT_KV, c.KVLR, akv_c)):
                            rn = pat.tile([1, c.CHUNK], F32, tag="rn")
                            nc.scalar.activation(
                                rn, ssq[:, cs], Act.Sqrt,
                                bias=eps_sb, scale=1.0 / denom)
                            rnr = pat.tile([1, c.CHUNK], F32, tag="rnr")
                            nc.vector.reciprocal(rnr, rn)
                            bc = pbc.tile([128, c.CHUNK], F32, tag="bc")
                            nc.tensor.matmul(
                                bc, ones_f, rnr, start=True, stop=True)
                            for t in range(ktn):
                                tgt = dst[:, t, cs]
                                nc.vector.scalar_tensor_tensor(
                                    out=tgt, in0=tgt,
                                    scalar=ln_sb[:, t:t + 1], in1=bc,
                                    op0=Alu.mult, op1=Alu.mult)
                        emit_rope(nc, pat, kpe[:, col:col + c.CHUNK],
                                  kperaw[:, cs],
                                  cos_sb[:, col:col + c.CHUNK],
                                  sin_sb[:, col:col + c.CHUNK], c.CHUNK)
                    sss = slice(sc * SUPER, (sc + 1) * SUPER)
                    nc.sync.dma_start(
                        out=aTs_r[:, 0:KT_Q, sss], in_=aq_c)
                    nc.sync.dma_start(
                        out=aTs_r[:, KT_Q:KT_Q + KT_KV, sss], in_=akv_c)

            # ------------- phase B: kv up-projection -------------------------
            pkv = top.enter_context(tc.tile_pool(name="pkv", bufs=1))
            knope = pkv.tile([128, H, c.S], BF, tag="knope")
            vv = pkv.tile([128, ST, H, c.DV + 1], BF, tag="vv")
            nc.vector.memset(vv[:, :, :, c.DV:], 1.0)

            with contextlib.ExitStack() as st:
                pbw = st.enter_context(tc.tile_pool(name="pbw", bufs=1))
                pbps = st.enter_context(
                    tc.tile_pool(name="pbps", bufs=3, space="PSUM"))
                wkv_sb = pbw.tile([128, KT_KV, KROWS + VCOLS], BF, tag="wkv")
                nc.sync.dma_start(out=wkv_sb, in_=w_kvb_r)
                akv_f = pbw.tile([128, KT_KV, c.S], BF, tag="akv_f")
                nc.sync.dma_start(
                    out=akv_f, in_=aTs_r[:, KT_Q:KT_Q + KT_KV, :])
                for mt in range(H):
                    for qc in range(NQC):
                        ps = pbps.tile([128, c.CHUNK], F32, tag="kps")
                        for kt in range(KT_KV):
                            nc.tensor.matmul(
                                ps, wkv_sb[:, kt, mt * 128:(mt + 1) * 128],
                                akv_f[:, kt, qc * c.CHUNK:(qc + 1) * c.CHUNK],
                                start=(kt == 0), stop=(kt == KT_KV - 1))
                        nc.scalar.copy(
                            knope[:, mt, qc * c.CHUNK:(qc + 1) * c.CHUNK], ps)
                vch = []
                v0 = 0
                while v0 < VCOLS:
                    vw = min(512, VCOLS - v0)
                    vch.append((v0, vw))
                    v0 += vw
                for stt in range(ST):
                    for v0, vw in vch:
                        ps = pbps.tile([128, 512], F32, tag="vps")
                        for kt in range(KT_KV):
                            nc.tensor.matmul(
                                ps[:, :vw],
                                akv_f[:, kt, stt * 128:(stt + 1) * 128],
                                wkv_sb[:, kt, KROWS + v0:KROWS + v0 + vw],
                                start=(kt == 0), stop=(kt == KT_KV - 1))
                        h0, hn = v0 // c.DV, vw // c.DV
                        nc.scalar.copy(
                            vv[:, stt, h0:h0 + hn, 0:c.DV],
                            ps[:, :vw].rearrange("p (h d) -> p h d", d=c.DV))

            # ------------- phase C: q up-projection + rope + spill -----------
            with contextlib.ExitStack() as st:
                pcq = st.enter_context(tc.tile_pool(name="pcq", bufs=1))
                pcw = st.enter_context(tc.tile_pool(name="pcw", bufs=2))
                pce = st.enter_context(tc.tile_pool(name="pce", bufs=3))
                pcps = st.enter_context(
                    tc.tile_pool(name="pcps", bufs=3, space="PSUM"))
                aq_f = pcq.tile([128, KT_Q, c.S], BF, tag="aq_f")
                nc.sync.dma_start(out=aq_f, in_=aTs_r[:, 0:KT_Q, :])
                for mt in range(MT_QN + MT_QP):
                    m0 = mt * 128
                    wq_sb = pcw.tile([128, KT_Q, 128], BF, tag="wq")
                    nc.sync.dma_start(out=wq_sb, in_=w_qb_r[:, :, m0:m0 + 128])
                    for qc in range(NQC):
                        col = qc * c.CHUNK
                        ps = pcps.tile([128, c.CHUNK], F32, tag="qps")
                        for kt in range(KT_Q):
                            nc.tensor.matmul(
                                ps, wq_sb[:, kt, :],
                                aq_f[:, kt, col:col + c.CHUNK],
                                start=(kt == 0), stop=(kt == KT_Q - 1))
                        qsb = pce.tile([128, c.CHUNK], BF, tag="qsb")
                        nc.scalar.mul(qsb, ps, SCALE)
                        if mt >= MT_QN:
                            roped = pce.tile([128, c.CHUNK], BF, tag="roped")
                            for j in (0, 1):
                                emit_rope(
                                    nc, pce,
                                    roped[j * 64:(j + 1) * 64],
                                    qsb[j * 64:(j + 1) * 64],
                                    cos_sb[:, col:col + c.CHUNK],
                                    sin_sb[:, col:col + c.CHUNK], c.CHUNK,
                                    p0=j * 64)
                            qsb = roped
                        nc.sync.dma_start(
                            out=qTs_ap[m0:m0 + 128, col:col + c.CHUNK],
                            in_=qsb)

            # ---------------- phase D: attention -----------------------------
            pot = top.enter_context(tc.tile_pool(name="pot", bufs=1))
            oT = pot.tile([128, H, c.S], BF, tag="oT")

            with contextlib.ExitStack() as st:
                pdq = st.enter_context(tc.tile_pool(name="pdq", bufs=2))
                pdp = st.enter_context(tc.tile_pool(name="pdp", bufs=2))
                pde = st.enter_context(tc.tile_pool(name="pde", bufs=4))
                pds = st.enter_context(
                    tc.tile_pool(name="pds", bufs=3, space="PSUM"))
                pdo = st.enter_context(
                    tc.tile_pool(name="pdo", bufs=2, space="PSUM"))
                pdt = st.enter_context(
                    tc.tile_pool(name="pdt", bufs=2, space="PSUM"))
                for h in range(H):
                    qn = pdq.tile([128, c.S], BF, tag="qn")
                    nc.sync.dma_start(
                        out=qn, in_=qTs_ap[h * 128:(h + 1) * 128, :])
                    qp = pdq.tile([c.DR, c.S], BF, tag="qp")
                    r0 = MT_QN * 128 + h * c.DR
                    nc.sync.dma_start(out=qp, in_=qTs_ap[r0:r0 + c.DR, :])
                    for qc in range(NQC):
                        col = qc * c.CHUNK
                        kmax = min(TPC * qc + TPC, ST)
                        probs = pdp.tile([128, ST, c.CHUNK], BF, tag="probs")
                        for kt in range(kmax):
                            ps = pds.tile([128, c.CHUNK], F32, tag="sc")
                            nc.tensor.matmul(
                                ps, knope[:, h, kt * 128:(kt + 1) * 128],
                                qn[:, col:col + c.CHUNK],
                                start=True, stop=False)
                            nc.tensor.matmul(
                                ps, kpe[:, kt * 128:(kt + 1) * 128],
                                qp[:, col:col + c.CHUNK],
                                start=False, stop=True)
                            nc.scalar.activation(
                                probs[:, kt, :], ps, Act.Exp)
                            d = kt - TPC * qc
                            if d >= 0:
                                nc.vector.tensor_tensor(
                                    out=probs[:, kt, :], in0=probs[:, kt, :],
                                    in1=mask_sb[:, d, :], op=Alu.mult)
                        for q2 in range(TPC):
                            qt = TPC * qc + q2
                            po = pdo.tile([128, c.DV + 1], F32, tag="po")
                            for kt in range(qt + 1):
                                nc.tensor.matmul(
                                    po, probs[:, kt, q2 * 128:(q2 + 1) * 128],
                                    vv[:, kt, h, :],
                                    start=(kt == 0), stop=(kt == qt))
                            rec = pde.tile([128, 1], F32, tag="rec")
                            nc.vector.reciprocal(rec, po[:, c.DV:c.DV + 1])
                            osb = pde.tile([128, c.DV], BF, tag="osb")
                            nc.scalar.mul(osb, po[:, :c.DV], rec)
                            pt = pdt.tile([128, 128], BF, tag="pt")
                            nc.tensor.transpose(pt, osb, ident)
                            nc.scalar.copy(
                                oT[:, h, qt * 128:(qt + 1) * 128], pt)

            # ---------------- phase E: o-projection --------------------------
            with contextlib.ExitStack() as st:
                pew = st.enter_context(tc.tile_pool(name="pew", bufs=2))
                peo = st.enter_context(tc.tile_pool(name="peo", bufs=3))
                peps = st.enter_context(
                    tc.tile_pool(name="peps", bufs=3, space="PSUM"))
                for mt in range(MT_O):
                    wo_sb = pew.tile([128, H, 128], BF, tag="wo")
                    nc.sync.dma_start(
                        out=wo_sb, in_=w_o_r[:, :, mt * 128:(mt + 1) * 128])
                    for qc in range(NQC):
                        col = qc * c.CHUNK
                        ps = peps.tile([128, c.CHUNK], F32, tag="ops")
                        for kt in range(H):
                            nc.tensor.matmul(
                                ps, wo_sb[:, kt, :],
                                oT[:, kt, col:col + c.CHUNK],
                                start=(kt == 0), stop=(kt == H - 1))
                        ob = peo.tile([128, c.CHUNK], F32, tag="ob")
                        nc.scalar.copy(ob, ps)
                        nc.sync.dma_start(
                            out=outT_ap[mt * 128:(mt + 1) * 128,
                                        col:col + c.CHUNK],
                            in_=ob)

    nc.compile()
    return nc


# ---------------------------------------------------------------------------
# host-side input preparation
# ---------------------------------------------------------------------------

def prep_shared(c: Cfg, w_a, q_ln_w, kv_ln_w):
    KT_Q = c.QLR // 128
    KT_KV = c.KVLR // 128
    TPC = c.CHUNK // 128
    half = c.PEH
    inv_freq = 1.0 / (c.THETA ** (np.arange(half, dtype=np.float32) / half))
    ang = np.arange(c.S, dtype=np.float32)[:, None] * inv_freq[None, :]
    cosT = np.ascontiguousarray(np.tile(np.cos(ang).T, (128 // half, 1))).astype(BF16)
    sinT = np.ascontiguousarray(np.tile(np.sin(ang).T, (128 // half, 1))).astype(BF16)
    k_idx = np.arange(128)[:, None]
    q_idx = np.arange(c.CHUNK)[None, :]
    maskm = np.stack(
        [(k_idx <= q_idx - 128 * d) for d in range(TPC)], axis=1
    ).astype(BF16)
    return {
        "w_a": np.ascontiguousarray(w_a).astype(BF16),
        "lnq": np.ascontiguousarray(
            q_ln_w.reshape(KT_Q, 128).T).astype(np.float32),
        "lnkv": np.ascontiguousarray(
            kv_ln_w.reshape(KT_KV, 128).T).astype(np.float32),
        "cosT": cosT,
        "sinT": sinT,
        "maskm": np.ascontiguousarray(maskm),
    }


def prep_group(c: Cfg, heads, w_qb, w_kvb, w_o, n_heads_total):
    """Reorganize the up-projection weights for one head group."""
    wq = w_qb.reshape(c.QLR, n_heads_total, c.DQK)[:, heads, :]
    wq_g = np.concatenate(
        [wq[:, :, :c.DN].reshape(c.QLR, -1), wq[:, :, c.DN:].reshape(c.QLR, -1)],
        axis=1)
    wkv = w_kvb.reshape(c.KVLR, n_heads_total, c.DN + c.DV)[:, heads, :]
    wkv_g = np.concatenate(
        [wkv[:, :, :c.DN].reshape(c.KVLR, -1),
         wkv[:, :, c.DN:].reshape(c.KVLR, -1)], axis=1)
    wo_g = w_o.reshape(n_heads_total, c.DV, c.HID)[heads].reshape(-1, c.HID)
    return {
        "w_qb": np.ascontiguousarray(wq_g).astype(BF16),
        "w_kvb": np.ascontiguousarray(wkv_g).astype(BF16),
        "w_o": np.ascontiguousarray(wo_g).astype(BF16),
    }


_PROGRAM = None


def _get_program():
    global _PROGRAM
    if _PROGRAM is None:
        _PROGRAM = build_program(FULL)
    return _PROGRAM


def kernel(x, w_a, q_ln_w, kv_ln_w, w_qb, w_kvb, w_o):
    from concourse.bass_utils import run_bass_kernel_spmd

    c = FULL
    x = np.asarray(x, dtype=np.float32)
    B = x.shape[0]
    n_heads = w_qb.shape[1] // c.DQK
    n_groups = n_heads // c.HPC
    assert B * n_groups == 8

    nc = _get_program()
    shared = prep_shared(c, np.asarray(w_a), np.asarray(q_ln_w),
                         np.asarray(kv_ln_w))
    groups = [
        prep_group(c, slice(g * c.HPC, (g + 1) * c.HPC), np.asarray(w_qb),
                   np.asarray(w_kvb), np.asarray(w_o), n_heads)
        for g in range(n_groups)
    ]
    xTs = [np.ascontiguousarray(x[b].T).astype(BF16) for b in range(B)]

    in_maps = []
    for core in range(8):
        b, g = divmod(core, n_groups)
        in_maps.append({"xT": xTs[b], **shared, **groups[g]})

    res = run_bass_kernel_spmd(nc, in_maps, core_ids=list(range(8)))
    outs = [r["outT"] for r in res.results]
    result = np.empty((B, c.S, c.HID), dtype=np.float32)
    for b in range(B):
        acc = outs[b * n_groups].copy()
        for g in range(1, n_groups):
            acc += outs[b * n_groups + g]
        result[b] = acc.T
    return result


# revision 11
# speedup vs baseline: 222.7021x; 222.7021x over previous
"""MLA (multi-head latent attention) forward kernel for Trainium2, 8 NeuronCores.

Sharding: 8 cores = 2 (batch) x 4 (head-groups of 10 heads).
Each core computes, for its batch b and its 10 heads:
  - 1/4 of the fused down-projection a = x @ w_a (sequence-sharded within the
    batch group, transposed-activation layout), rmsnorm + k_pe rope on its
    slice, then an AllGather of the normalized aT across the 4 cores
  - q/kv up-projections for its heads, causal attention, and the partial
    o-projection (w_o rows of its heads).  Host sums the 4 partials per batch.

Device layout notes:
  - activations are kept transposed ([feature, seq]) so weights act as the
    stationary lhsT operand of the PE in their natural [in, out] orientation.
  - attention computes scoresT [keys, q]; softmax runs without max-subtraction
    (scores are bounded by construction), masking is a binary multiply on the
    exp'd probabilities, and sum-of-exp comes from a ones-column appended to V
    in the AV matmul.  Per-row 1/sum is applied on PSUM eviction.
  - normalized aT (gathered) and qT round-trip through DRAM so SBUF tile-pool
    lifetimes nest properly (pool releases must be LIFO).
"""

import math
import sys
from dataclasses import dataclass

if "/opt/trn_rl_repo" not in sys.path:
    sys.path.insert(0, "/opt/trn_rl_repo")

import ml_dtypes
import numpy as np

BF16 = ml_dtypes.bfloat16


@dataclass(frozen=True)
class Cfg:
    HID: int = 5120
    S: int = 2048
    QLR: int = 1536
    KVLR: int = 512
    DN: int = 128
    DR: int = 64
    DV: int = 128
    HPC: int = 10          # heads per core
    CHUNK: int = 512       # q-position chunk (PSUM bank width)
    GS: int = 1            # cores per batch group (sequence-shard of phase A)
    NCORES: int = 8
    EPS: float = 1e-6
    THETA: float = 10000.0

    @property
    def DQK(self):
        return self.DN + self.DR

    @property
    def PEH(self):
        return self.DR // 2

    @property
    def SL(self):
        return self.S // self.GS


FULL = Cfg(GS=4)


def build_program(c: Cfg):
    import contextlib

    import concourse.bass as bass  # noqa: F401
    import concourse.mybir as mybir
    import concourse.tile as tile
    from concourse import bacc
    from concourse.masks import make_identity

    dt = mybir.dt
    BF = dt.bfloat16
    F32 = dt.float32
    Alu = mybir.AluOpType
    Act = mybir.ActivationFunctionType

    KT_HID = c.HID // 128
    KT_Q = c.QLR // 128
    KT_KV = c.KVLR // 128
    NQC = c.S // c.CHUNK
    GS = c.GS
    SL = c.SL
    NLC = SL // c.CHUNK             # local q-chunks in phase A
    ST = c.S // 128
    H = c.HPC
    TPC = c.CHUNK // 128            # 128-tiles per chunk (4)
    QROWS = H * (c.DN + c.DR)
    MT_QN = H * c.DN // 128
    MT_QP = H * c.DR // 128
    KROWS = H * c.DN
    VCOLS = H * c.DV
    OROWS = H * c.DV
    MT_O = c.HID // 128
    ACOLS = c.QLR + c.KVLR + c.DR
    AT_PAD = (ACOLS + 127) // 128   # padded row-tiles of the gather buffer
    SCALE = 1.0 / math.sqrt(c.DQK)

    assert c.DN == 128 and c.DV == 128 and c.DR == 64 and H % 2 == 0
    assert SL % c.CHUNK == 0

    nc = bacc.Bacc("TRN2", num_devices=(c.NCORES if GS > 1 else None))
    xT = nc.dram_tensor("xT", [c.HID, SL], BF, kind="ExternalInput")
    w_a = nc.dram_tensor("w_a", [c.HID, ACOLS], BF, kind="ExternalInput")
    w_qb = nc.dram_tensor("w_qb", [c.QLR, QROWS], BF, kind="ExternalInput")
    w_kvb = nc.dram_tensor("w_kvb", [c.KVLR, KROWS + VCOLS], BF, kind="ExternalInput")
    w_o = nc.dram_tensor("w_o", [OROWS, c.HID], BF, kind="ExternalInput")
    cosT = nc.dram_tensor("cosT", [128, c.S], BF, kind="ExternalInput")
    sinT = nc.dram_tensor("sinT", [128, c.S], BF, kind="ExternalInput")
    cosA = nc.dram_tensor("cosA", [128, SL], BF, kind="ExternalInput")
    sinA = nc.dram_tensor("sinA", [128, SL], BF, kind="ExternalInput")
    lnq = nc.dram_tensor("lnq", [128, KT_Q], F32, kind="ExternalInput")
    lnkv = nc.dram_tensor("lnkv", [128, KT_KV], F32, kind="ExternalInput")
    maskm = nc.dram_tensor("maskm", [128, TPC, c.CHUNK], BF, kind="ExternalInput")
    outT = nc.dram_tensor("outT", [c.HID, c.S], F32, kind="ExternalOutput")
    qTs = nc.dram_tensor("qTs", [QROWS, c.S], BF, kind="Internal")
    agl = nc.dram_tensor("agl", [AT_PAD * 128, SL], BF, kind="Internal")
    if GS > 1:
        agg = nc.dram_tensor("agg", [GS * AT_PAD * 128, SL], BF, kind="Internal")
    else:
        agg = agl

    xT_r = xT.ap().rearrange("(t p) s -> p t s", p=128)
    w_a_r = w_a.ap().rearrange("(t p) m -> p t m", p=128)
    w_qb_r = w_qb.ap().rearrange("(t p) m -> p t m", p=128)
    w_kvb_r = w_kvb.ap().rearrange("(t p) m -> p t m", p=128)
    w_o_r = w_o.ap().rearrange("(t p) m -> p t m", p=128)
    agl_r = agl.ap().rearrange("(t p) s -> p t s", p=128)
    agg_r = agg.ap().rearrange("(g t p) s -> g p t s", g=GS, p=128)
    qTs_ap = qTs.ap()
    outT_ap = outT.ap()

    def emit_rope(nc, pool, dst64, src64, cos_ap, sin_ap, W, p0=0):
        # cos_ap/sin_ap are [128, W] (table replicated every PEH partitions);
        # slices are taken at each operand's base partition because DVE
        # tensor_tensor requires equal base partitions for SBUF inputs.
        ph = c.PEH
        t1, t2 = src64[0:ph], src64[ph:2 * ph]
        d1, d2 = dst64[0:ph], dst64[ph:2 * ph]
        c1, s1 = cos_ap[p0:p0 + ph], sin_ap[p0:p0 + ph]
        c2, s2 = cos_ap[p0 + ph:p0 + 2 * ph], sin_ap[p0 + ph:p0 + 2 * ph]
        ra = pool.tile([ph, W], F32, tag="rope_a", name="rope_a")
        rb = pool.tile([ph, W], F32, tag="rope_b", name="rope_b")
        nc.vector.tensor_tensor(out=ra, in0=t1, in1=c1, op=Alu.mult)
        nc.vector.tensor_tensor(out=rb, in0=t2, in1=s2, op=Alu.mult)
        nc.vector.tensor_tensor(out=d1, in0=ra, in1=rb, op=Alu.subtract)
        nc.vector.tensor_tensor(out=ra, in0=t2, in1=c2, op=Alu.mult)
        nc.vector.tensor_tensor(out=rb, in0=t1, in1=s1, op=Alu.mult)
        nc.vector.tensor_tensor(out=d2, in0=ra, in1=rb, op=Alu.add)

    with tile.TileContext(nc, pool_alloc_mode="queue") as tc:
        with contextlib.ExitStack() as top:
            pers = top.enter_context(tc.tile_pool(name="pers", bufs=1))
            cos_sb = pers.tile([128, c.S], BF, tag="cos_sb")
            sin_sb = pers.tile([128, c.S], BF, tag="sin_sb")
            cosa_sb = pers.tile([128, SL], BF, tag="cosa_sb")
            sina_sb = pers.tile([128, SL], BF, tag="sina_sb")
            lnq_sb = pers.tile([128, KT_Q], F32, tag="lnq_sb")
            lnkv_sb = pers.tile([128, KT_KV], F32, tag="lnkv_sb")
            mask_sb = pers.tile([128, TPC, c.CHUNK], BF, tag="mask_sb")
            ident = pers.tile([128, 128], BF, tag="ident")
            ones_f = pers.tile([1, 128], F32, tag="ones_f")
            ones_c = pers.tile([128, 1], BF, tag="ones_c")
            eps_sb = pers.tile([1, 1], F32, tag="eps_sb")
            nc.vector.memset(eps_sb, c.EPS)
            kpe = pers.tile([c.DR, c.S], BF, tag="kpe")
            nc.sync.dma_start(out=cos_sb, in_=cosT.ap())
            nc.sync.dma_start(out=sin_sb, in_=sinT.ap())
            nc.sync.dma_start(out=cosa_sb, in_=cosA.ap())
            nc.sync.dma_start(out=sina_sb, in_=sinA.ap())
            nc.sync.dma_start(out=lnq_sb, in_=lnq.ap())
            nc.sync.dma_start(out=lnkv_sb, in_=lnkv.ap())
            nc.sync.dma_start(out=mask_sb, in_=maskm.ap())
            make_identity(nc, ident)
            nc.vector.memset(ones_f, 1.0)
            nc.vector.memset(ones_c, 1.0)

            # -------- phase A: local a-proj + rmsnorm + k_pe rope + gather ---
            with contextlib.ExitStack() as st:
                pax = st.enter_context(tc.tile_pool(name="pax", bufs=1))
                paw = st.enter_context(tc.tile_pool(name="paw", bufs=2))
                pat = st.enter_context(tc.tile_pool(name="pat", bufs=2))
                paa = st.enter_context(tc.tile_pool(name="paa", bufs=1))
                paps = st.enter_context(
                    tc.tile_pool(name="paps", bufs=2, space="PSUM"))
                pssq = st.enter_context(
                    tc.tile_pool(name="pssq", bufs=1, space="PSUM"))
                pbc = st.enter_context(
                    tc.tile_pool(name="pbc", bufs=2, space="PSUM"))

                mtiles = ([("q", i) for i in range(KT_Q)]
                          + [("kv", i) for i in range(KT_KV)]
                          + [("pe", 0)])
                x_sb = pax.tile([128, KT_HID, SL], BF, tag="x_sb")
                nc.sync.dma_start(out=x_sb, in_=xT_r)
                aq_c = paa.tile([128, KT_Q, SL], BF, tag="aq_c")
                akv_c = paa.tile([128, KT_KV, SL], BF, tag="akv_c")
                ssq_q = pssq.tile([1, SL], F32, tag="ssq_q")
                ssq_kv = pssq.tile([1, SL], F32, tag="ssq_kv")
                kperaw = pat.tile([c.DR, SL], BF, tag="kperaw", bufs=1)
                kpel = pat.tile([c.DR, SL], BF, tag="kpel", bufs=1)
                for seg, ti in mtiles:
                    if seg == "q":
                        m0, mw = ti * 128, 128
                    elif seg == "kv":
                        m0, mw = c.QLR + ti * 128, 128
                    else:
                        m0, mw = c.QLR + c.KVLR, c.DR
                    wa_sb = paw.tile([128, KT_HID, 128], BF, tag="wa_sb")
                    nc.sync.dma_start(
                        out=wa_sb[:, :, :mw], in_=w_a_r[:, :, m0:m0 + mw])
                    for qi in range(NLC):
                        cs = slice(qi * c.CHUNK, (qi + 1) * c.CHUNK)
                        ps = paps.tile([128, c.CHUNK], F32, tag="aps")
                        for kt in range(KT_HID):
                            nc.tensor.matmul(
                                ps[:mw], wa_sb[:, kt, :mw],
                                x_sb[:, kt, cs],
                                start=(kt == 0), stop=(kt == KT_HID - 1))
                        if seg == "q":
                            nc.scalar.copy(aq_c[:, ti, cs], ps)
                            sq = pat.tile([128, c.CHUNK], BF, tag="sq",
                                          bufs=3)
                            nc.scalar.square(sq, ps)
                            nc.tensor.matmul(
                                ssq_q[:, cs], ones_c, sq,
                                start=(ti == 0), stop=(ti == KT_Q - 1))
                        elif seg == "kv":
                            nc.scalar.copy(akv_c[:, ti, cs], ps)
                            sq = pat.tile([128, c.CHUNK], BF, tag="sq",
                                          bufs=3)
                            nc.scalar.square(sq, ps)
                            nc.tensor.matmul(
                                ssq_kv[:, cs], ones_c, sq,
                                start=(ti == 0), stop=(ti == KT_KV - 1))
                        else:
                            nc.scalar.copy(kperaw[:, cs], ps[:mw])
                # normalize + rope + spill
                for qi in range(NLC):
                    cs = slice(qi * c.CHUNK, (qi + 1) * c.CHUNK)
                    for seg, ssq, ln_sb, ktn, denom, dst in (
                            ("q", ssq_q, lnq_sb, KT_Q, c.QLR, aq_c),
                            ("kv", ssq_kv, lnkv_sb, KT_KV, c.KVLR, akv_c)):
                        rn = pat.tile([1, c.CHUNK], F32, tag="rn")
                        nc.scalar.activation(
                            rn, ssq[:, cs], Act.Sqrt,
                            bias=eps_sb, scale=1.0 / denom)
                        rnr = pat.tile([1, c.CHUNK], F32, tag="rnr")
                        nc.vector.reciprocal(rnr, rn)
                        bc = pbc.tile([128, c.CHUNK], F32, tag="bc")
                        nc.tensor.matmul(
                            bc, ones_f, rnr, start=True, stop=True)
                        for t in range(ktn):
                            tgt = dst[:, t, cs]
                            nc.vector.scalar_tensor_tensor(
                                out=tgt, in0=tgt,
                                scalar=ln_sb[:, t:t + 1], in1=bc,
                                op0=Alu.mult, op1=Alu.mult)
                    emit_rope(nc, pat, kpel[:, cs], kperaw[:, cs],
                              cosa_sb[:, cs], sina_sb[:, cs], c.CHUNK)
                nc.sync.dma_start(out=agl_r[:, 0:KT_Q, :], in_=aq_c)
                nc.sync.dma_start(
                    out=agl_r[:, KT_Q:KT_Q + KT_KV, :], in_=akv_c)
                nc.sync.dma_start(
                    out=agl_r[0:c.DR, KT_Q + KT_KV, :], in_=kpel)

            if GS > 1:
                groups = [[b * GS + j for j in range(GS)]
                          for b in range(c.NCORES // GS)]
                nc.gpsimd.collective_compute(
                    "AllGather", mybir.AluOpType.bypass,
                    replica_groups=groups,
                    ins=[agl.ap()], outs=[agg.ap()])
            for g in range(GS):
                nc.sync.dma_start(
                    out=kpe[:, g * SL:(g + 1) * SL],
                    in_=agg_r[g, 0:c.DR, KT_Q + KT_KV, :])

            # ------------- phase B: kv up-projection -------------------------
            pkv = top.enter_context(tc.tile_pool(name="pkv", bufs=1))
            knope = pkv.tile([128, H, c.S], BF, tag="knope")
            vv = pkv.tile([128, ST, H, c.DV + 1], BF, tag="vv")
            nc.vector.memset(vv[:, :, :, c.DV:], 1.0)

            with contextlib.ExitStack() as st:
                pbw = st.enter_context(tc.tile_pool(name="pbw", bufs=1))
                pbps = st.enter_context(
                    tc.tile_pool(name="pbps", bufs=3, space="PSUM"))
                wkv_sb = pbw.tile([128, KT_KV, KROWS + VCOLS], BF, tag="wkv")
                nc.sync.dma_start(out=wkv_sb, in_=w_kvb_r)
                akv_f = pbw.tile([128, KT_KV, c.S], BF, tag="akv_f")
                for g in range(GS):
                    nc.sync.dma_start(
                        out=akv_f[:, :, g * SL:(g + 1) * SL],
                        in_=agg_r[g, :, KT_Q:KT_Q + KT_KV, :])
                for mt in range(H):
                    for qc in range(NQC):
                        ps = pbps.tile([128, c.CHUNK], F32, tag="kps")
                        for kt in range(KT_KV):
                            nc.tensor.matmul(
                                ps, wkv_sb[:, kt, mt * 128:(mt + 1) * 128],
                                akv_f[:, kt, qc * c.CHUNK:(qc + 1) * c.CHUNK],
                                start=(kt == 0), stop=(kt == KT_KV - 1))
                        nc.scalar.copy(
                            knope[:, mt, qc * c.CHUNK:(qc + 1) * c.CHUNK], ps)
                vch = []
                v0 = 0
                while v0 < VCOLS:
                    vw = min(512, VCOLS - v0)
                    vch.append((v0, vw))
                    v0 += vw
                for stt in range(ST):
                    for v0, vw in vch:
                        ps = pbps.tile([128, 512], F32, tag="vps")
                        for kt in range(KT_KV):
                            nc.tensor.matmul(
                                ps[:, :vw],
                                akv_f[:, kt, stt * 128:(stt + 1) * 128],
                                wkv_sb[:, kt, KROWS + v0:KROWS + v0 + vw],
                                start=(kt == 0), stop=(kt == KT_KV - 1))
                        h0, hn = v0 // c.DV, vw // c.DV
                        nc.scalar.copy(
                            vv[:, stt, h0:h0 + hn, 0:c.DV],
                            ps[:, :vw].rearrange("p (h d) -> p h d", d=c.DV))

            # ------------- phase C: q up-projection + rope + spill -----------
            with contextlib.ExitStack() as st:
                pcq = st.enter_context(tc.tile_pool(name="pcq", bufs=1))
                pcw = st.enter_context(tc.tile_pool(name="pcw", bufs=2))
                pce = st.enter_context(tc.tile_pool(name="pce", bufs=3))
                pcps = st.enter_context(
                    tc.tile_pool(name="pcps", bufs=3, space="PSUM"))
                aq_f = pcq.tile([128, KT_Q, c.S], BF, tag="aq_f")
                for g in range(GS):
                    nc.sync.dma_start(
                        out=aq_f[:, :, g * SL:(g + 1) * SL],
                        in_=agg_r[g, :, 0:KT_Q, :])
                for mt in range(MT_QN + MT_QP):
                    m0 = mt * 128
                    wq_sb = pcw.tile([128, KT_Q, 128], BF, tag="wq")
                    nc.sync.dma_start(out=wq_sb, in_=w_qb_r[:, :, m0:m0 + 128])
                    for qc in range(NQC):
                        col = qc * c.CHUNK
                        ps = pcps.tile([128, c.CHUNK], F32, tag="qps")
                        for kt in range(KT_Q):
                            nc.tensor.matmul(
                                ps, wq_sb[:, kt, :],
                                aq_f[:, kt, col:col + c.CHUNK],
                                start=(kt == 0), stop=(kt == KT_Q - 1))
                        qsb = pce.tile([128, c.CHUNK], BF, tag="qsb")
                        nc.scalar.mul(qsb, ps, SCALE)
                        if mt >= MT_QN:
                            roped = pce.tile([128, c.CHUNK], BF, tag="roped")
                            for j in (0, 1):
                                emit_rope(
                                    nc, pce,
                                    roped[j * 64:(j + 1) * 64],
                                    qsb[j * 64:(j + 1) * 64],
                                    cos_sb[:, col:col + c.CHUNK],
                                    sin_sb[:, col:col + c.CHUNK], c.CHUNK,
                                    p0=j * 64)
                            qsb = roped
                        nc.sync.dma_start(
                            out=qTs_ap[m0:m0 + 128, col:col + c.CHUNK],
                            in_=qsb)

            # ---------------- phase D: attention -----------------------------
            pot = top.enter_context(tc.tile_pool(name="pot", bufs=1))
            oT = pot.tile([128, H, c.S], BF, tag="oT")

            with contextlib.ExitStack() as st:
                pdq = st.enter_context(tc.tile_pool(name="pdq", bufs=2))
                pdp = st.enter_context(tc.tile_pool(name="pdp", bufs=1))
                pde = st.enter_context(tc.tile_pool(name="pde", bufs=4))
                pds = st.enter_context(
                    tc.tile_pool(name="pds", bufs=3, space="PSUM"))
                pdo = st.enter_context(
                    tc.tile_pool(name="pdo", bufs=2, space="PSUM"))
                pdt = st.enter_context(
                    tc.tile_pool(name="pdt", bufs=2, space="PSUM"))
                for h in range(H):
                    qn = pdq.tile([128, c.S], BF, tag="qn")
                    nc.sync.dma_start(
                        out=qn, in_=qTs_ap[h * 128:(h + 1) * 128, :])
                    qp = pdq.tile([c.DR, c.S], BF, tag="qp")
                    r0 = MT_QN * 128 + h * c.DR
                    nc.sync.dma_start(out=qp, in_=qTs_ap[r0:r0 + c.DR, :])
                    for qc in range(NQC):
                        col = qc * c.CHUNK
                        kmax = min(TPC * qc + TPC, ST)
                        probs = pdp.tile([128, ST, c.CHUNK], BF, tag="probs")
                        for kt in range(kmax):
                            ps = pds.tile([128, c.CHUNK], F32, tag="sc")
                            nc.tensor.matmul(
                                ps, knope[:, h, kt * 128:(kt + 1) * 128],
                                qn[:, col:col + c.CHUNK],
                                start=True, stop=False)
                            nc.tensor.matmul(
                                ps, kpe[:, kt * 128:(kt + 1) * 128],
                                qp[:, col:col + c.CHUNK],
                                start=False, stop=True)
                            nc.scalar.activation(
                                probs[:, kt, :], ps, Act.Exp)
                            d = kt - TPC * qc
                            if d >= 0:
                                nc.vector.tensor_tensor(
                                    out=probs[:, kt, :], in0=probs[:, kt, :],
                                    in1=mask_sb[:, d, :], op=Alu.mult)
                        for q2 in range(TPC):
                            qt = TPC * qc + q2
                            po = pdo.tile([128, c.DV + 1], F32, tag="po")
                            for kt in range(qt + 1):
                                nc.tensor.matmul(
                                    po, probs[:, kt, q2 * 128:(q2 + 1) * 128],
                                    vv[:, kt, h, :],
                                    start=(kt == 0), stop=(kt == qt))
                            rec = pde.tile([128, 1], F32, tag="rec")
                            nc.vector.reciprocal(rec, po[:, c.DV:c.DV + 1])
                            osb = pde.tile([128, c.DV], BF, tag="osb")
                            nc.scalar.mul(osb, po[:, :c.DV], rec)
                            pt = pdt.tile([128, 128], BF, tag="pt")
                            nc.tensor.transpose(pt, osb, ident)
                            nc.scalar.copy(
                                oT[:, h, qt * 128:(qt + 1) * 128], pt)

            # ---------------- phase E: o-projection --------------------------
            with contextlib.ExitStack() as st:
                pew = st.enter_context(tc.tile_pool(name="pew", bufs=2))
                peo = st.enter_context(tc.tile_pool(name="peo", bufs=3))
                peps = st.enter_context(
                    tc.tile_pool(name="peps", bufs=3, space="PSUM"))
                for mt in range(MT_O):
                    wo_sb = pew.tile([128, H, 128], BF, tag="wo")
                    nc.sync.dma_start(
                        out=wo_sb, in_=w_o_r[:, :, mt * 128:(mt + 1) * 128])
                    for qc in range(NQC):
                        col = qc * c.CHUNK
                        ps = peps.tile([128, c.CHUNK], F32, tag="ops")
                        for kt in range(H):
                            nc.tensor.matmul(
                                ps, wo_sb[:, kt, :],
                                oT[:, kt, col:col + c.CHUNK],
                                start=(kt == 0), stop=(kt == H - 1))
                        ob = peo.tile([128, c.CHUNK], F32, tag="ob")
                        nc.scalar.copy(ob, ps)
                        nc.sync.dma_start(
                            out=outT_ap[mt * 128:(mt + 1) * 128,
                                        col:col + c.CHUNK],
                            in_=ob)

    nc.compile()
    return nc


# ---------------------------------------------------------------------------
# host-side input preparation
# ---------------------------------------------------------------------------

def prep_shared(c: Cfg, w_a, q_ln_w, kv_ln_w):
    KT_Q = c.QLR // 128
    KT_KV = c.KVLR // 128
    TPC = c.CHUNK // 128
    half = c.PEH
    inv_freq = 1.0 / (c.THETA ** (np.arange(half, dtype=np.float32) / half))
    ang = np.arange(c.S, dtype=np.float32)[:, None] * inv_freq[None, :]
    cosT = np.ascontiguousarray(
        np.tile(np.cos(ang).T, (128 // half, 1))).astype(BF16)
    sinT = np.ascontiguousarray(
        np.tile(np.sin(ang).T, (128 // half, 1))).astype(BF16)
    k_idx = np.arange(128)[:, None]
    q_idx = np.arange(c.CHUNK)[None, :]
    maskm = np.stack(
        [(k_idx <= q_idx - 128 * d) for d in range(TPC)], axis=1
    ).astype(BF16)
    return {
        "w_a": np.ascontiguousarray(w_a).astype(BF16),
        "lnq": np.ascontiguousarray(
            q_ln_w.reshape(KT_Q, 128).T).astype(np.float32),
        "lnkv": np.ascontiguousarray(
            kv_ln_w.reshape(KT_KV, 128).T).astype(np.float32),
        "cosT": cosT,
        "sinT": sinT,
        "maskm": np.ascontiguousarray(maskm),
    }


def prep_group(c: Cfg, heads, w_qb, w_kvb, w_o, n_heads_total):
    """Reorganize the up-projection weights for one head group."""
    wq = w_qb.reshape(c.QLR, n_heads_total, c.DQK)[:, heads, :]
    wq_g = np.concatenate(
        [wq[:, :, :c.DN].reshape(c.QLR, -1), wq[:, :, c.DN:].reshape(c.QLR, -1)],
        axis=1)
    wkv = w_kvb.reshape(c.KVLR, n_heads_total, c.DN + c.DV)[:, heads, :]
    wkv_g = np.concatenate(
        [wkv[:, :, :c.DN].reshape(c.KVLR, -1),
         wkv[:, :, c.DN:].reshape(c.KVLR, -1)], axis=1)
    wo_g = w_o.reshape(n_heads_total, c.DV, c.HID)[heads].reshape(-1, c.HID)
    return {
        "w_qb": np.ascontiguousarray(wq_g).astype(BF16),
        "w_kvb": np.ascontiguousarray(wkv_g).astype(BF16),
        "w_o": np.ascontiguousarray(wo_g).astype(BF16),
    }


_PROGRAM = None


def _get_program():
    global _PROGRAM
    if _PROGRAM is None:
        _PROGRAM = build_program(FULL)
    return _PROGRAM


def kernel(x, w_a, q_ln_w, kv_ln_w, w_qb, w_kvb, w_o):
    from concourse.bass_utils import run_bass_kernel_spmd

    c = FULL
    x = np.asarray(x, dtype=np.float32)
    B = x.shape[0]
    n_heads = w_qb.shape[1] // c.DQK
    n_groups = n_heads // c.HPC
    assert B * n_groups == c.NCORES and n_groups == c.GS

    nc = _get_program()
    shared = prep_shared(c, np.asarray(w_a), np.asarray(q_ln_w),
                         np.asarray(kv_ln_w))
    groups = [
        prep_group(c, slice(g * c.HPC, (g + 1) * c.HPC), np.asarray(w_qb),
                   np.asarray(w_kvb), np.asarray(w_o), n_heads)
        for g in range(n_groups)
    ]
    xTs = [np.ascontiguousarray(x[b].T).astype(BF16) for b in range(B)]

    in_maps = []
    for core in range(c.NCORES):
        b, g = divmod(core, n_groups)
        sl = slice(g * c.SL, (g + 1) * c.SL)
        in_maps.append({
            "xT": np.ascontiguousarray(xTs[b][:, sl]),
            "cosA": np.ascontiguousarray(shared["cosT"][:, sl]),
            "sinA": np.ascontiguousarray(shared["sinT"][:, sl]),
            **shared, **groups[g],
        })

    res = run_bass_kernel_spmd(nc, in_maps, core_ids=list(range(c.NCORES)))
    outs = [r["outT"] for r in res.results]
    result = np.empty((B, c.S, c.HID), dtype=np.float32)
    for b in range(B):
        acc = outs[b * n_groups].copy()
        for g in range(1, n_groups):
            acc += outs[b * n_groups + g]
        result[b] = acc.T
    return result
